# revision 1
# baseline (speedup 1.0000x reference)
"""Trainium2 Bass kernel for AngleConvCat GNN message passing.

Computation (see reference):
    total = concat([vertex_feat[j_idx], edge_feat[k_idx], edge_feat[i_idx],
                    angle_feat], axis=-1)                     # [N_ANGLES, 1024]
    core  = silu(BN_train(total @ W_core))                    # [N_ANGLES, 256]
    gate  = sigmoid(BN_train(total @ W_gate))
    out   = core * gate + angle_feat

Distribution: data-parallel over the angle dimension across 8 NeuronCores.
Tables (vertex/edge) + weights replicated; BN batch stats (per-feature
sum/sumsq) all-reduced across cores.

Device dataflow per core (angles padded to PAD rows; tiles of 512 rows):
  Phase 1 (per tile):
    - indirect-DMA gather of vertex/edge rows (bf16) -> row-major [128,256]
    - transpose via TensorE (matmul against identity) -> feature-major
    - angle features arrive pre-transposed from host (bf16)
    - 2x (core/gate) matmul [1024x256 weights, K accumulated in PSUM]
    - PSUM -> SBUF bf16 cast, bn_stats partial stats, spill to DRAM
  AllReduce of per-feature sum/sumsq (one [128,8] f32 collective);
  BN scale/bias computed on-chip.
  Phase 2 (per tile): reload spill, sigmoid/silu via ScalarE with fused
  per-partition affine, gating product, residual add, write bf16 output
  in feature-major layout; host transposes back and casts to f32.

Padding uses an appended all-zero row in each table and zero angle rows, so
padded rows contribute exactly zero to the BN sums (stats divide by the true
N on-chip).
"""

import numpy as np

# ---------------------------------------------------------------------------
# Problem constants (hardcoded per harness contract)
# ---------------------------------------------------------------------------
N_ATOMS = 50000
N_EDGES = 400000
N_ANGLES = 300000
D = 256                      # feature dim of each component
IN_DIM = 1024
N_CORES = 8
SHARD = N_ANGLES // N_CORES  # 37500
ROW_TILE = 512               # rows per super-tile (matmul moving free dim)
SUB = 128                    # rows per gather subtile (one per partition)
BN_EPS = 1e-5


def _import_concourse():
    try:
        import concourse  # noqa: F401
    except ImportError:
        import sys
        for p in ("/opt/trn_rl_repo", "/root/.axon_site/_ro/trn_rl_repo"):
            if p not in sys.path:
                sys.path.insert(0, p)
        import concourse  # noqa: F401


# ---------------------------------------------------------------------------
# Graph builder
# ---------------------------------------------------------------------------
def build_graph(pad_rows, n_atoms_tbl, n_edges_tbl, total_n, evac_split=2,
                xg_bufs=6, xt_bufs=3, sp_bufs=8, p2_bufs=3):
    """Builds the SPMD Bass graph. pad_rows must be divisible by ROW_TILE.

    n_atoms_tbl/n_edges_tbl: table row counts INCLUDING the zero pad row.
    total_n: true global row count for BN statistics (sum over all cores).
    """
    _import_concourse()
    from concourse import bass, bacc, mybir, tile
    from concourse.masks import make_identity

    dt = mybir.dt
    f32, bf16, i32 = dt.float32, dt.bfloat16, dt.int32
    AF = mybir.ActivationFunctionType
    ALU = mybir.AluOpType

    assert pad_rows % ROW_TILE == 0
    NT = pad_rows // ROW_TILE          # super-tiles
    NSUB = pad_rows // SUB             # subtiles
    SUBS_PER_TILE = ROW_TILE // SUB    # 4

    nc = bacc.Bacc("TRN2", target_bir_lowering=False, debug=False,
                   num_devices=N_CORES)

    vertex = nc.dram_tensor("vertex", [n_atoms_tbl, D], bf16, kind="ExternalInput")
    edge = nc.dram_tensor("edge", [n_edges_tbl, D], bf16, kind="ExternalInput")
    angle_t = nc.dram_tensor("angle_t", [2, 128, pad_rows], bf16, kind="ExternalInput")
    idx_j = nc.dram_tensor("idx_j", [128, NSUB], i32, kind="ExternalInput")
    idx_k = nc.dram_tensor("idx_k", [128, NSUB], i32, kind="ExternalInput")
    idx_i = nc.dram_tensor("idx_i", [128, NSUB], i32, kind="ExternalInput")
    w = nc.dram_tensor("w", [IN_DIM, 512], bf16, kind="ExternalInput")
    gamma = nc.dram_tensor("gamma", [128, 4], f32, kind="ExternalInput")
    beta = nc.dram_tensor("beta", [128, 4], f32, kind="ExternalInput")
    out = nc.dram_tensor("out", [2, 128, pad_rows], bf16, kind="ExternalOutput")

    with tile.TileContext(nc) as tc:
        with (
            tc.tile_pool(name="const", bufs=1) as constp,
            tc.tile_pool(name="stats", bufs=1) as statsp,
            tc.tile_pool(name="xg", bufs=xg_bufs) as xgp,
            tc.tile_pool(name="xt", bufs=xt_bufs) as xtp,
            tc.tile_pool(name="sp", bufs=sp_bufs) as spp,
            tc.tile_pool(name="p2", bufs=p2_bufs) as p2p,
            tc.tile_pool(name="tp_psum", bufs=2, space="PSUM") as tpp,
            tc.tile_pool(name="mm_psum", bufs=4, space="PSUM") as mmp,
            tc.tile_pool(name="dram", bufs=1, space="DRAM") as dramp,
        ):
            # ---------------- constants ----------------
            ident = constp.tile([128, 128], bf16)
            make_identity(nc, ident[:])

            w_sb = constp.tile([128, 8, 512], bf16)
            for k in range(8):
                nc.sync.dma_start(w_sb[:, k, :], w[k * 128:(k + 1) * 128, :])

            idxj_sb = constp.tile([128, NSUB], i32)
            idxk_sb = constp.tile([128, NSUB], i32)
            idxi_sb = constp.tile([128, NSUB], i32)
            nc.sync.dma_start(idxj_sb[:], idx_j[:, :])
            nc.sync.dma_start(idxk_sb[:], idx_k[:, :])
            nc.sync.dma_start(idxi_sb[:], idx_i[:, :])

            gam_sb = constp.tile([128, 4], f32)
            bet_sb = constp.tile([128, 4], f32)
            nc.sync.dma_start(gam_sb[:], gamma[:, :])
            nc.sync.dma_start(bet_sb[:], beta[:, :])

            # per-tile bn_stats go here: [128, m(4), NT*6]
            stats_sb = statsp.tile([128, 4, NT * 6], f32)

            spill = dramp.tile([NT, 4, 128, ROW_TILE], bf16)

            gathers = ((idxj_sb, vertex), (idxk_sb, edge), (idxi_sb, edge))

            # ---------------- phase 1 ----------------
            for t in range(NT):
                xt = xtp.tile([128, 8, ROW_TILE], bf16, tag="xt")
                # angle chunks come pre-transposed from DRAM
                for m in range(2):
                    nc.sync.dma_start(
                        xt[:, 6 + m, :],
                        angle_t[m, :, t * ROW_TILE:(t + 1) * ROW_TILE])
                for s in range(SUBS_PER_TILE):
                    sub = t * SUBS_PER_TILE + s
                    xg = xgp.tile([128, 3, D], bf16, tag="xg")
                    for gi, (idx_sb, table) in enumerate(gathers):
                        nc.gpsimd.indirect_dma_start(
                            out=xg[:, gi, :],
                            out_offset=None,
                            in_=table[:, :],
                            in_offset=bass.IndirectOffsetOnAxis(
                                ap=idx_sb[:, sub:sub + 1], axis=0),
                        )
                    # transpose the 6 gathered 128x128 blocks via TensorE
                    tp = tpp.tile([128, 768], f32, tag="tp")
                    for c in range(6):
                        nc.tensor.matmul(
                            tp[:, c * 128:(c + 1) * 128],
                            lhsT=xg[:, c // 2, (c % 2) * 128:(c % 2) * 128 + 128],
                            rhs=ident[:],
                            start=True, stop=True)
                    # evacuate PSUM -> feature-major bf16 X^T
                    src = tp[:].rearrange("p (c r) -> p c r", c=6)
                    dst = xt[:, 0:6, s * 128:(s + 1) * 128]
                    if s % evac_split == 0:
                        nc.vector.tensor_copy(dst, src)
                    else:
                        nc.scalar.copy(dst, src)

                # core/gate matmuls: m = (core0, core1, gate0, gate1)
                for m in range(4):
                    ps = mmp.tile([128, ROW_TILE], f32, tag="mm")
                    for k in range(8):
                        nc.tensor.matmul(
                            ps[:],
                            lhsT=w_sb[:, k, m * 128:(m + 1) * 128],
                            rhs=xt[:, k, :],
                            start=(k == 0), stop=(k == 7))
                    sp = spp.tile([128, ROW_TILE], bf16, tag="sp")
                    if m % 2 == 0:
                        nc.scalar.copy(sp[:], ps[:])
                    else:
                        nc.vector.tensor_copy(sp[:], ps[:])
                    nc.vector.bn_stats(stats_sb[:, m, t * 6:(t + 1) * 6], sp[:])
                    nc.sync.dma_start(spill[t, m], sp[:])

            # ---------------- stats + collective ----------------
            agg = statsp.tile([128, 4, 2], f32)       # (mean, var) per m
            for m in range(4):
                nc.vector.bn_aggr(agg[:, m, :], stats_sb[:, m, :])

            cc_sb = statsp.tile([128, 8], f32)        # sums(4) | sumsqs(4)
            tmp = statsp.tile([128, 4], f32)
            means = agg[:, :, 0]
            variances = agg[:, :, 1]
            # sum = mean * pad_rows
            nc.vector.tensor_scalar(cc_sb[:, 0:4], means, float(pad_rows), None,
                                    op0=ALU.mult)
            # sumsq = (var + mean^2) * pad_rows
            nc.vector.tensor_tensor(tmp[:], means, means, op=ALU.mult)
            nc.vector.tensor_tensor(tmp[:], tmp[:], variances, op=ALU.add)
            nc.vector.tensor_scalar(cc_sb[:, 4:8], tmp[:], float(pad_rows), None,
                                    op0=ALU.mult)

            cc_in = dramp.tile([128, 8], f32)
            cc_out = dramp.tile([128, 8], f32)
            nc.gpsimd.dma_start(cc_in[:], cc_sb[:])
            nc.gpsimd.collective_compute(
                "AllReduce", ALU.add,
                replica_groups=[list(range(N_CORES))],
                ins=[cc_in.opt()],
                outs=[cc_out.opt()],
            )
            gstat = statsp.tile([128, 8], f32)
            nc.sync.dma_start(gstat[:], cc_out[:])

            # mean/var -> scale/bias
            mean_g = statsp.tile([128, 4], f32)
            vpe = statsp.tile([128, 4], f32)
            scale_sb = statsp.tile([128, 4], f32)
            bias_sb = statsp.tile([128, 4], f32)
            t1 = statsp.tile([128, 4], f32)
            inv_n = 1.0 / float(total_n)
            nc.vector.tensor_scalar(mean_g[:], gstat[:, 0:4], inv_n, None,
                                    op0=ALU.mult)
            nc.vector.tensor_scalar(vpe[:], gstat[:, 4:8], inv_n, None,
                                    op0=ALU.mult)        # E[x^2]
            nc.vector.tensor_tensor(t1[:], mean_g[:], mean_g[:], op=ALU.mult)
            nc.vector.tensor_tensor(vpe[:], vpe[:], t1[:], op=ALU.subtract)
            nc.vector.tensor_scalar(vpe[:], vpe[:], BN_EPS, None, op0=ALU.add)
            # rsqrt(vpe): reciprocal (DVE) -> sqrt (ACT) -> one Newton polish
            nc.vector.reciprocal(t1[:], vpe[:])
            s0 = statsp.tile([128, 4], f32)
            nc.scalar.activation(s0[:], t1[:], AF.Sqrt)
            # s1 = s0 * (1.5 - 0.5 * vpe * s0^2)
            nc.vector.tensor_tensor(t1[:], s0[:], s0[:], op=ALU.mult)
            nc.vector.tensor_tensor(t1[:], t1[:], vpe[:], op=ALU.mult)
            nc.vector.tensor_scalar(t1[:], t1[:], -0.5, 1.5, op0=ALU.mult,
                                    op1=ALU.add)
            nc.vector.tensor_tensor(s0[:], s0[:], t1[:], op=ALU.mult)
            # scale = gamma * rsqrt; bias = beta - mean * scale
            nc.vector.tensor_tensor(scale_sb[:], gam_sb[:], s0[:], op=ALU.mult)
            nc.vector.tensor_tensor(t1[:], mean_g[:], scale_sb[:], op=ALU.mult)
            nc.vector.tensor_tensor(bias_sb[:], bet_sb[:], t1[:], op=ALU.subtract)

            # ---------------- phase 2 ----------------
            for t in range(NT):
                ld = p2p.tile([128, 4, ROW_TILE], bf16, tag="ld")
                nc.sync.dma_start(ld[:], spill[t].rearrange("m p c -> p m c"))
                ang = p2p.tile([128, 2, ROW_TILE], bf16, tag="ang")
                nc.sync.dma_start(
                    ang[:],
                    angle_t[:, :, t * ROW_TILE:(t + 1) * ROW_TILE]
                    .rearrange("m p c -> p m c"))

                sig = p2p.tile([128, 4, ROW_TILE], bf16, tag="sig")
                for m in range(4):
                    nc.scalar.activation(
                        sig[:, m, :], ld[:, m, :], AF.Sigmoid,
                        bias=bias_sb[:, m:m + 1], scale=scale_sb[:, m:m + 1])
                xc = p2p.tile([128, 2, ROW_TILE], bf16, tag="xc")
                prod = p2p.tile([128, 2, ROW_TILE], bf16, tag="prod")
                outt = p2p.tile([128, 2, ROW_TILE], bf16, tag="outt")
                for m in range(2):
                    # xc = scale*x + bias  (pre-activation for silu = xc*sig(xc))
                    nc.vector.tensor_scalar(
                        xc[:, m, :], ld[:, m, :],
                        scale_sb[:, m:m + 1], bias_sb[:, m:m + 1],
                        op0=ALU.mult, op1=ALU.add)
                    nc.vector.tensor_tensor(prod[:, m, :], xc[:, m, :],
                                            sig[:, m, :], op=ALU.mult)
                    nc.vector.tensor_tensor(prod[:, m, :], prod[:, m, :],
                                            sig[:, 2 + m, :], op=ALU.mult)
                    nc.vector.tensor_tensor(outt[:, m, :], prod[:, m, :],
                                            ang[:, m, :], op=ALU.add)
                nc.sync.dma_start(
                    out[:, :, t * ROW_TILE:(t + 1) * ROW_TILE]
                    .rearrange("m p c -> p m c"),
                    outt[:])

    nc.compile()
    return nc


# ---------------------------------------------------------------------------
# Host-side prep
# ---------------------------------------------------------------------------
def prepare_in_maps(vertex_feat, edge_feat, angle_feat, k_idx, j_idx, i_idx,
                    W_core, W_gate, gamma_c, beta_c, gamma_g, beta_g,
                    n_cores=N_CORES, pad_rows=None):
    import ml_dtypes
    bf16 = ml_dtypes.bfloat16

    n_angles = angle_feat.shape[0]
    shard = n_angles // n_cores
    if pad_rows is None:
        pad_rows = ((shard + ROW_TILE - 1) // ROW_TILE) * ROW_TILE
    nsub = pad_rows // SUB
    n_atoms = vertex_feat.shape[0]
    n_edges = edge_feat.shape[0]

    vertex_b = np.zeros((n_atoms + 1, D), dtype=bf16)
    vertex_b[:n_atoms] = vertex_feat.astype(bf16)
    edge_b = np.zeros((n_edges + 1, D), dtype=bf16)
    edge_b[:n_edges] = edge_feat.astype(bf16)

    w_fused = np.concatenate(
        [np.asarray(W_core), np.asarray(W_gate)], axis=1).astype(bf16)

    gam = np.stack([gamma_c[0:128], gamma_c[128:256],
                    gamma_g[0:128], gamma_g[128:256]], axis=1).astype(np.float32)
    bet = np.stack([beta_c[0:128], beta_c[128:256],
                    beta_g[0:128], beta_g[128:256]], axis=1).astype(np.float32)

    def prep_idx(idx, pad_val):
        idx = np.asarray(idx, dtype=np.int64)
        out = np.full((n_cores, pad_rows), pad_val, dtype=np.int32)
        out[:, :shard] = idx.reshape(n_cores, shard)
        # [pad_rows] -> [128 partitions, nsub] (position-in-subtile major)
        return [np.ascontiguousarray(out[c].reshape(nsub, SUB).T)
                for c in range(n_cores)]

    idx_j_l = prep_idx(j_idx, n_atoms)
    idx_k_l = prep_idx(k_idx, n_edges)
    idx_i_l = prep_idx(i_idx, n_edges)

    angle_f32 = np.asarray(angle_feat, dtype=np.float32)
    in_maps = []
    for c in range(n_cores):
        ang = np.zeros((pad_rows, D), dtype=np.float32)
        ang[:shard] = angle_f32[c * shard:(c + 1) * shard]
        ang_t = np.ascontiguousarray(ang.T).reshape(2, 128, pad_rows).astype(bf16)
        in_maps.append({
            "vertex": vertex_b,
            "edge": edge_b,
            "angle_t": ang_t,
            "idx_j": idx_j_l[c],
            "idx_k": idx_k_l[c],
            "idx_i": idx_i_l[c],
            "w": w_fused,
            "gamma": gam,
            "beta": bet,
        })
    return in_maps, pad_rows, shard


def assemble_output(results, shard, pad_rows, n_cores=N_CORES):
    """results: list (per core) of dict with 'out' [2,128,pad_rows] bf16."""
    full = np.empty((n_cores * shard, D), dtype=np.float32)
    for c in range(n_cores):
        o = np.asarray(results[c]["out"]).astype(np.float32)
        o = o.reshape(D, pad_rows)          # feature f = m*128+p
        full[c * shard:(c + 1) * shard] = o[:, :shard].T
    return full


# ---------------------------------------------------------------------------
# Entry point
# ---------------------------------------------------------------------------
_GRAPH_CACHE = {}


def _get_graph(pad_rows, n_atoms_tbl, n_edges_tbl, total_n):
    key = (pad_rows, n_atoms_tbl, n_edges_tbl, total_n)
    if key not in _GRAPH_CACHE:
        _GRAPH_CACHE[key] = build_graph(pad_rows, n_atoms_tbl, n_edges_tbl,
                                        total_n)
    return _GRAPH_CACHE[key]


def kernel(vertex_feat, edge_feat, angle_feat, edge_index, k_idx, j_idx, i_idx,
           W_core, W_gate, gamma_c, beta_c, gamma_g, beta_g, _trace=False):
    _import_concourse()
    from concourse.bass_utils import run_bass_kernel_spmd

    vertex_feat = np.asarray(vertex_feat)
    edge_feat = np.asarray(edge_feat)
    angle_feat = np.asarray(angle_feat)

    in_maps, pad_rows, shard = prepare_in_maps(
        vertex_feat, edge_feat, angle_feat, k_idx, j_idx, i_idx,
        W_core, W_gate, gamma_c, beta_c, gamma_g, beta_g)

    nc = _get_graph(pad_rows, vertex_feat.shape[0] + 1, edge_feat.shape[0] + 1,
                    angle_feat.shape[0])

    res = run_bass_kernel_spmd(nc, in_maps, core_ids=list(range(N_CORES)),
                               trace=_trace)
    out = assemble_output(res.results, shard, pad_rows)
    if _trace:
        kernel.last_exec_time_ns = res.exec_time_ns
        kernel.last_results = res
    return out


# revision 2
# speedup vs baseline: 1.5538x; 1.5538x over previous
"""Trainium2 Bass kernel for AngleConvCat GNN message passing.

Computation (see reference):
    total = concat([vertex_feat[j_idx], edge_feat[k_idx], edge_feat[i_idx],
                    angle_feat], axis=-1)                     # [N_ANGLES, 1024]
    core  = silu(BN_train(total @ W_core))                    # [N_ANGLES, 256]
    gate  = sigmoid(BN_train(total @ W_gate))
    out   = core * gate + angle_feat

Distribution: data-parallel over the angle dimension across 8 NeuronCores.
Tables (vertex/edge) + weights replicated; BN batch stats (per-feature
sum/sumsq) all-reduced across cores.

Device dataflow per core (angles padded to PAD rows; tiles of 512 rows):
  Phase 1 (per tile):
    - indirect-DMA gather of vertex/edge rows (bf16) -> row-major [128,256]
    - transpose via TensorE (matmul against identity) -> feature-major
    - angle features arrive pre-transposed from host (bf16)
    - 2x (core/gate) matmul [1024x256 weights, K accumulated in PSUM]
    - PSUM -> SBUF bf16 cast, bn_stats partial stats, spill to DRAM
  AllReduce of per-feature sum/sumsq (one [128,8] f32 collective);
  BN scale/bias computed on-chip.
  Phase 2 (per tile): reload spill, sigmoid/silu via ScalarE with fused
  per-partition affine, gating product, residual add, write bf16 output
  in feature-major layout; host transposes back and casts to f32.

Padding uses an appended all-zero row in each table and zero angle rows, so
padded rows contribute exactly zero to the BN sums (stats divide by the true
N on-chip).
"""

import numpy as np

# ---------------------------------------------------------------------------
# Problem constants (hardcoded per harness contract)
# ---------------------------------------------------------------------------
N_ATOMS = 50000
N_EDGES = 400000
N_ANGLES = 300000
D = 256                      # feature dim of each component
IN_DIM = 1024
N_CORES = 8
SHARD = N_ANGLES // N_CORES  # 37500
ROW_TILE = 512               # rows per super-tile (matmul moving free dim)
SUB = 128                    # rows per gather subtile (one per partition)
BN_EPS = 1e-5


def _import_concourse():
    try:
        import concourse  # noqa: F401
    except ImportError:
        import sys
        for p in ("/opt/trn_rl_repo", "/root/.axon_site/_ro/trn_rl_repo"):
            if p not in sys.path:
                sys.path.insert(0, p)
        import concourse  # noqa: F401


# ---------------------------------------------------------------------------
# Graph builder
# ---------------------------------------------------------------------------
def build_graph(pad_rows, n_atoms_tbl, n_edges_tbl, total_n, evac_split=2,
                xg_bufs=6, xt_bufs=3, sp_bufs=8, p2_bufs=3, gather_batch=1,
                timing_loop=0):
    """Builds the SPMD Bass graph. pad_rows must be divisible by ROW_TILE.

    n_atoms_tbl/n_edges_tbl: table row counts INCLUDING the zero pad row.
    total_n: true global row count for BN statistics (sum over all cores).
    gather_batch: gathered rows per indirect DMA per partition (1, 2 or 4).
    timing_loop: if >0, build a timing variant — the whole per-tile body
      (phase1+phase2) wrapped in a device For_i loop of that many reps, with
      the collective skipped (scale=gamma, bias=beta).
    """
    _import_concourse()
    from concourse import bass, bacc, mybir, tile
    from concourse.masks import make_identity

    dt = mybir.dt
    f32, bf16, i32 = dt.float32, dt.bfloat16, dt.int32
    AF = mybir.ActivationFunctionType
    ALU = mybir.AluOpType

    assert pad_rows % ROW_TILE == 0
    NT = pad_rows // ROW_TILE          # super-tiles
    NSUB = pad_rows // SUB             # subtiles
    SUBS_PER_TILE = ROW_TILE // SUB    # 4
    assert SUBS_PER_TILE % gather_batch == 0

    nc = bacc.Bacc("TRN2", target_bir_lowering=False, debug=False,
                   num_devices=N_CORES)

    vertex = nc.dram_tensor("vertex", [n_atoms_tbl, D], bf16, kind="ExternalInput")
    edge = nc.dram_tensor("edge", [n_edges_tbl, D], bf16, kind="ExternalInput")
    angle_t = nc.dram_tensor("angle_t", [2, 128, pad_rows], bf16, kind="ExternalInput")
    idx_j = nc.dram_tensor("idx_j", [128, NSUB], i32, kind="ExternalInput")
    idx_k = nc.dram_tensor("idx_k", [128, NSUB], i32, kind="ExternalInput")
    idx_i = nc.dram_tensor("idx_i", [128, NSUB], i32, kind="ExternalInput")
    w = nc.dram_tensor("w", [IN_DIM, 512], bf16, kind="ExternalInput")
    gamma = nc.dram_tensor("gamma", [128, 4], f32, kind="ExternalInput")
    beta = nc.dram_tensor("beta", [128, 4], f32, kind="ExternalInput")
    out = nc.dram_tensor("out", [2, 128, pad_rows], bf16, kind="ExternalOutput")

    with tile.TileContext(nc) as tc:
        with (
            tc.tile_pool(name="const", bufs=1) as constp,
            tc.tile_pool(name="stats", bufs=1) as statsp,
            tc.tile_pool(name="xg", bufs=xg_bufs) as xgp,
            tc.tile_pool(name="xt", bufs=xt_bufs) as xtp,
            tc.tile_pool(name="sp", bufs=sp_bufs) as spp,
            tc.tile_pool(name="p2", bufs=p2_bufs) as p2p,
            tc.tile_pool(name="tp_psum", bufs=2, space="PSUM") as tpp,
            tc.tile_pool(name="mm_psum", bufs=4, space="PSUM") as mmp,
            tc.tile_pool(name="dram", bufs=1, space="DRAM") as dramp,
        ):
            # ---------------- constants ----------------
            ident = constp.tile([128, 128], bf16)
            make_identity(nc, ident[:])

            w_sb = constp.tile([128, 8, 512], bf16)
            for k in range(8):
                nc.sync.dma_start(w_sb[:, k, :], w[k * 128:(k + 1) * 128, :])

            idxj_sb = constp.tile([128, NSUB], i32)
            idxk_sb = constp.tile([128, NSUB], i32)
            idxi_sb = constp.tile([128, NSUB], i32)
            nc.sync.dma_start(idxj_sb[:], idx_j[:, :])
            nc.sync.dma_start(idxk_sb[:], idx_k[:, :])
            nc.sync.dma_start(idxi_sb[:], idx_i[:, :])

            gam_sb = constp.tile([128, 4], f32)
            bet_sb = constp.tile([128, 4], f32)
            nc.sync.dma_start(gam_sb[:], gamma[:, :])
            nc.sync.dma_start(bet_sb[:], beta[:, :])

            # per-tile bn_stats go here: [128, m(4), NT*6]
            stats_sb = statsp.tile([128, 4, NT * 6], f32)

            spill = dramp.tile([NT, 4, 128, ROW_TILE], bf16)

            gathers = ((idxj_sb, vertex), (idxk_sb, edge), (idxi_sb, edge))

            # ---------------- phase 1 (per super-tile) ----------------
            def emit_phase1_tile(t):
                xt = xtp.tile([128, 8, ROW_TILE], bf16, tag="xt")
                # angle chunks come pre-transposed from DRAM
                for m in range(2):
                    nc.sync.dma_start(
                        xt[:, 6 + m, :],
                        angle_t[m, :, t * ROW_TILE:(t + 1) * ROW_TILE])
                gb = gather_batch
                for s0 in range(0, SUBS_PER_TILE, gb):
                    sub = t * SUBS_PER_TILE + s0
                    xg = xgp.tile([128, gb, 3, D], bf16, tag="xg")
                    for gi, (idx_sb, table) in enumerate(gathers):
                        nc.gpsimd.indirect_dma_start(
                            out=xg[:, :, gi, :],
                            out_offset=None,
                            in_=table[:, :],
                            in_offset=bass.IndirectOffsetOnAxis(
                                ap=idx_sb[:, sub:sub + gb], axis=0),
                        )
                    for s in range(s0, s0 + gb):
                        # transpose the 6 gathered 128x128 blocks via TensorE
                        tp = tpp.tile([128, 768], f32, tag="tp")
                        for c in range(6):
                            nc.tensor.matmul(
                                tp[:, c * 128:(c + 1) * 128],
                                lhsT=xg[:, s - s0, c // 2,
                                        (c % 2) * 128:(c % 2) * 128 + 128],
                                rhs=ident[:],
                                start=True, stop=True)
                        # evacuate PSUM -> feature-major bf16 X^T
                        src = tp[:].rearrange("p (c r) -> p c r", c=6)
                        dst = xt[:, 0:6, s * 128:(s + 1) * 128]
                        if s % evac_split == 0:
                            nc.vector.tensor_copy(dst, src)
                        else:
                            nc.scalar.copy(dst, src)

                # core/gate matmuls: m = (core0, core1, gate0, gate1)
                for m in range(4):
                    ps = mmp.tile([128, ROW_TILE], f32, tag="mm")
                    for k in range(8):
                        nc.tensor.matmul(
                            ps[:],
                            lhsT=w_sb[:, k, m * 128:(m + 1) * 128],
                            rhs=xt[:, k, :],
                            start=(k == 0), stop=(k == 7))
                    sp = spp.tile([128, ROW_TILE], bf16, tag="sp")
                    if m % 2 == 0:
                        nc.scalar.copy(sp[:], ps[:])
                    else:
                        nc.vector.tensor_copy(sp[:], ps[:])
                    nc.vector.bn_stats(stats_sb[:, m, t * 6:(t + 1) * 6], sp[:])
                    nc.sync.dma_start(spill[t, m], sp[:])

            # ---------------- stats + collective ----------------
            def emit_stats_cc():
                agg = statsp.tile([128, 4, 2], f32)       # (mean, var) per m
                for m in range(4):
                    nc.vector.bn_aggr(agg[:, m, :], stats_sb[:, m, :])

                cc_sb = statsp.tile([128, 8], f32)        # sums(4) | sumsqs(4)
                tmp = statsp.tile([128, 4], f32)
                means = agg[:, :, 0]
                variances = agg[:, :, 1]
                # sum = mean * pad_rows
                nc.vector.tensor_scalar(cc_sb[:, 0:4], means, float(pad_rows),
                                        None, op0=ALU.mult)
                # sumsq = (var + mean^2) * pad_rows
                nc.vector.tensor_tensor(tmp[:], means, means, op=ALU.mult)
                nc.vector.tensor_tensor(tmp[:], tmp[:], variances, op=ALU.add)
                nc.vector.tensor_scalar(cc_sb[:, 4:8], tmp[:], float(pad_rows),
                                        None, op0=ALU.mult)

                cc_in = dramp.tile([128, 8], f32)
                cc_out = dramp.tile([128, 8], f32)
                nc.gpsimd.dma_start(cc_in[:], cc_sb[:])
                nc.gpsimd.collective_compute(
                    "AllReduce", ALU.add,
                    replica_groups=[list(range(N_CORES))],
                    ins=[cc_in.opt()],
                    outs=[cc_out.opt()],
                )
                gstat = statsp.tile([128, 8], f32)
                nc.sync.dma_start(gstat[:], cc_out[:])

                # mean/var -> scale/bias
                mean_g = statsp.tile([128, 4], f32)
                vpe = statsp.tile([128, 4], f32)
                scale_sb = statsp.tile([128, 4], f32)
                bias_sb = statsp.tile([128, 4], f32)
                t1 = statsp.tile([128, 4], f32)
                inv_n = 1.0 / float(total_n)
                nc.vector.tensor_scalar(mean_g[:], gstat[:, 0:4], inv_n, None,
                                        op0=ALU.mult)
                nc.vector.tensor_scalar(vpe[:], gstat[:, 4:8], inv_n, None,
                                        op0=ALU.mult)        # E[x^2]
                nc.vector.tensor_tensor(t1[:], mean_g[:], mean_g[:], op=ALU.mult)
                nc.vector.tensor_tensor(vpe[:], vpe[:], t1[:], op=ALU.subtract)
                nc.vector.tensor_scalar(vpe[:], vpe[:], BN_EPS, None, op0=ALU.add)
                # rsqrt(vpe): reciprocal (DVE) -> sqrt (ACT) -> Newton polish
                nc.vector.reciprocal(t1[:], vpe[:])
                s0 = statsp.tile([128, 4], f32)
                nc.scalar.activation(s0[:], t1[:], AF.Sqrt)
                # s1 = s0 * (1.5 - 0.5 * vpe * s0^2)
                nc.vector.tensor_tensor(t1[:], s0[:], s0[:], op=ALU.mult)
                nc.vector.tensor_tensor(t1[:], t1[:], vpe[:], op=ALU.mult)
                nc.vector.tensor_scalar(t1[:], t1[:], -0.5, 1.5, op0=ALU.mult,
                                        op1=ALU.add)
                nc.vector.tensor_tensor(s0[:], s0[:], t1[:], op=ALU.mult)
                # scale = gamma * rsqrt; bias = beta - mean * scale
                nc.vector.tensor_tensor(scale_sb[:], gam_sb[:], s0[:],
                                        op=ALU.mult)
                nc.vector.tensor_tensor(t1[:], mean_g[:], scale_sb[:],
                                        op=ALU.mult)
                nc.vector.tensor_tensor(bias_sb[:], bet_sb[:], t1[:],
                                        op=ALU.subtract)
                return scale_sb, bias_sb

            # ---------------- phase 2 (per super-tile) ----------------
            def emit_phase2_tile(t, scale_sb, bias_sb):
                ld = p2p.tile([128, 4, ROW_TILE], bf16, tag="ld")
                nc.sync.dma_start(ld[:], spill[t].rearrange("m p c -> p m c"))
                ang = p2p.tile([128, 2, ROW_TILE], bf16, tag="ang")
                nc.sync.dma_start(
                    ang[:],
                    angle_t[:, :, t * ROW_TILE:(t + 1) * ROW_TILE]
                    .rearrange("m p c -> p m c"))

                sig = p2p.tile([128, 4, ROW_TILE], bf16, tag="sig")
                for m in range(4):
                    nc.scalar.activation(
                        sig[:, m, :], ld[:, m, :], AF.Sigmoid,
                        bias=bias_sb[:, m:m + 1], scale=scale_sb[:, m:m + 1])
                xc = p2p.tile([128, 2, ROW_TILE], bf16, tag="xc")
                prod = p2p.tile([128, 2, ROW_TILE], bf16, tag="prod")
                outt = p2p.tile([128, 2, ROW_TILE], bf16, tag="outt")
                for m in range(2):
                    # xc = scale*x + bias (pre-activation; silu = xc*sig(xc))
                    nc.vector.tensor_scalar(
                        xc[:, m, :], ld[:, m, :],
                        scale_sb[:, m:m + 1], bias_sb[:, m:m + 1],
                        op0=ALU.mult, op1=ALU.add)
                    nc.vector.tensor_tensor(prod[:, m, :], xc[:, m, :],
                                            sig[:, m, :], op=ALU.mult)
                    nc.vector.tensor_tensor(prod[:, m, :], prod[:, m, :],
                                            sig[:, 2 + m, :], op=ALU.mult)
                    nc.vector.tensor_tensor(outt[:, m, :], prod[:, m, :],
                                            ang[:, m, :], op=ALU.add)
                nc.sync.dma_start(
                    out[:, :, t * ROW_TILE:(t + 1) * ROW_TILE]
                    .rearrange("m p c -> p m c"),
                    outt[:])

            if timing_loop:
                def body(_i=None):
                    for t in range(NT):
                        emit_phase1_tile(t)
                    for t in range(NT):
                        emit_phase2_tile(t, gam_sb, bet_sb)
                with tc.For_i(0, timing_loop, 1):
                    body()
            else:
                for t in range(NT):
                    emit_phase1_tile(t)
                scale_sb, bias_sb = emit_stats_cc()
                for t in range(NT):
                    emit_phase2_tile(t, scale_sb, bias_sb)

    nc.compile()
    return nc


# ---------------------------------------------------------------------------
# Host-side prep
# ---------------------------------------------------------------------------
def prepare_in_maps(vertex_feat, edge_feat, angle_feat, k_idx, j_idx, i_idx,
                    W_core, W_gate, gamma_c, beta_c, gamma_g, beta_g,
                    n_cores=N_CORES, pad_rows=None):
    import ml_dtypes
    bf16 = ml_dtypes.bfloat16

    n_angles = angle_feat.shape[0]
    shard = n_angles // n_cores
    if pad_rows is None:
        pad_rows = ((shard + ROW_TILE - 1) // ROW_TILE) * ROW_TILE
    nsub = pad_rows // SUB
    n_atoms = vertex_feat.shape[0]
    n_edges = edge_feat.shape[0]

    vertex_b = np.zeros((n_atoms + 1, D), dtype=bf16)
    vertex_b[:n_atoms] = vertex_feat.astype(bf16)
    edge_b = np.zeros((n_edges + 1, D), dtype=bf16)
    edge_b[:n_edges] = edge_feat.astype(bf16)

    w_fused = np.concatenate(
        [np.asarray(W_core), np.asarray(W_gate)], axis=1).astype(bf16)

    gam = np.stack([gamma_c[0:128], gamma_c[128:256],
                    gamma_g[0:128], gamma_g[128:256]], axis=1).astype(np.float32)
    bet = np.stack([beta_c[0:128], beta_c[128:256],
                    beta_g[0:128], beta_g[128:256]], axis=1).astype(np.float32)

    def prep_idx(idx, pad_val):
        idx = np.asarray(idx, dtype=np.int64)
        out = np.full((n_cores, pad_rows), pad_val, dtype=np.int32)
        out[:, :shard] = idx.reshape(n_cores, shard)
        # [pad_rows] -> [128 partitions, nsub] (position-in-subtile major)
        return [np.ascontiguousarray(out[c].reshape(nsub, SUB).T)
                for c in range(n_cores)]

    idx_j_l = prep_idx(j_idx, n_atoms)
    idx_k_l = prep_idx(k_idx, n_edges)
    idx_i_l = prep_idx(i_idx, n_edges)

    angle_f32 = np.asarray(angle_feat, dtype=np.float32)
    in_maps = []
    for c in range(n_cores):
        ang = np.zeros((pad_rows, D), dtype=np.float32)
        ang[:shard] = angle_f32[c * shard:(c + 1) * shard]
        ang_t = np.ascontiguousarray(ang.T).reshape(2, 128, pad_rows).astype(bf16)
        in_maps.append({
            "vertex": vertex_b,
            "edge": edge_b,
            "angle_t": ang_t,
            "idx_j": idx_j_l[c],
            "idx_k": idx_k_l[c],
            "idx_i": idx_i_l[c],
            "w": w_fused,
            "gamma": gam,
            "beta": bet,
        })
    return in_maps, pad_rows, shard


def assemble_output(results, shard, pad_rows, n_cores=N_CORES):
    """results: list (per core) of dict with 'out' [2,128,pad_rows] bf16."""
    full = np.empty((n_cores * shard, D), dtype=np.float32)
    for c in range(n_cores):
        o = np.asarray(results[c]["out"]).astype(np.float32)
        o = o.reshape(D, pad_rows)          # feature f = m*128+p
        full[c * shard:(c + 1) * shard] = o[:, :shard].T
    return full


# ---------------------------------------------------------------------------
# Entry point
# ---------------------------------------------------------------------------
_GRAPH_CACHE = {}


def _get_graph(pad_rows, n_atoms_tbl, n_edges_tbl, total_n):
    key = (pad_rows, n_atoms_tbl, n_edges_tbl, total_n)
    if key not in _GRAPH_CACHE:
        _GRAPH_CACHE[key] = build_graph(pad_rows, n_atoms_tbl, n_edges_tbl,
                                        total_n)
    return _GRAPH_CACHE[key]


def kernel(vertex_feat, edge_feat, angle_feat, edge_index, k_idx, j_idx, i_idx,
           W_core, W_gate, gamma_c, beta_c, gamma_g, beta_g, _trace=False):
    _import_concourse()
    from concourse.bass_utils import run_bass_kernel_spmd

    vertex_feat = np.asarray(vertex_feat)
    edge_feat = np.asarray(edge_feat)
    angle_feat = np.asarray(angle_feat)

    in_maps, pad_rows, shard = prepare_in_maps(
        vertex_feat, edge_feat, angle_feat, k_idx, j_idx, i_idx,
        W_core, W_gate, gamma_c, beta_c, gamma_g, beta_g)

    nc = _get_graph(pad_rows, vertex_feat.shape[0] + 1, edge_feat.shape[0] + 1,
                    angle_feat.shape[0])

    res = run_bass_kernel_spmd(nc, in_maps, core_ids=list(range(N_CORES)),
                               trace=_trace)
    out = assemble_output(res.results, shard, pad_rows)
    if _trace:
        kernel.last_exec_time_ns = res.exec_time_ns
        kernel.last_results = res
    return out
